# revision 1
# baseline (speedup 1.0000x reference)
"""Trainium2 Bass kernel for nn_Attention (general-score attention energies +
softmax over the batch axis).

Math (reference):
    proj     = einsum('lbh,oh->lbo', enc, W) + b      # [L, B, H]
    energies = einsum('bh,lbh->bl', hidden, proj)     # [B, L]
    attn     = softmax(energies, axis=0)[:, None, :]  # [B, 1, L]

Algebraic rewrite used here:
    energies[b, l] = (hidden @ W)[b] . enc[l, b] + hidden[b] . b
This removes the O(L*B*H*H) projection matmul entirely; the kernel is a
memory-bound stream over enc (256 MB) with a tiny [B,H]x[H,H] matmul up front.

Distribution: enc is sharded along L across 8 cores (128 l-values per core).
The softmax is over the batch axis (per l), so every core's softmax is fully
local -- no collectives. hidden / W / b are replicated.

Per-core dataflow:
  - u = hidden @ W and c = hidden . b on TensorE. hidden^T arrives
    pre-transposed from the host with its columns duplicated, so the
    matmul output covers all 128 PSUM partitions (both (l-parity, b)
    halves) at the same PE cost -- no on-chip transposes, no dup copies.
  - stream enc in 2 MiB tiles [128 part = (l-pair, b), 4x1024 free]; one
    fused DVE scalar_tensor_tensor per [128, 1024] block computes
    ecol = sum_h enc * u in a single pass (accum_out); bias c added once
    at the end ([128, 64] tensor_scalar add).
  - softmax over b: PE transpose [64,64] halves -> reduce_max(negate) ->
    ScalarE exp(+bias) with fused row-sum -> reciprocal -> scale
  - PE transpose back, interleave even/odd l, DMA out [64, 128]

Timing notes (HW, neuron-profile): ~121-134 us NEFF exec (run-to-run HBM
jitter); DMA-bound: 36.4 MB/core at ~360 GB/s with a ~30 us setup phase
(W DMA + 16 serial fp32 matmuls) fully overlapped with enc prefetch.
"""

import numpy as np

import concourse.bass as bass
import concourse.bacc as bacc
import concourse.tile as tile
from concourse import mybir
from concourse.bass_utils import run_bass_kernel_spmd

F32 = mybir.dt.float32

B = 64          # batch
H = 1024        # hidden dim
L = 1024        # enc_len
NCORES = 8
LS = L // NCORES            # 128 l-values per core
TILE_L = 8                  # l-values per DMA tile (2 MiB per tile)
NT = LS // TILE_L           # 16 stream tiles per core
QK = TILE_L // 2            # 4 [128, 1024] blocks per tile
MULT = mybir.AluOpType.mult
ADD = mybir.AluOpType.add


def build_program() -> bacc.Bacc:
    nc = bacc.Bacc(
        "TRN2", target_bir_lowering=False, debug=False, num_devices=NCORES
    )

    # "setup" packs [hidT duplicated to both b-halves | bvT | identity]:
    #   cols 0:1024   hidden^T chunks: [:, 128k:128k+128] = lhsT for o-chunk k
    #                 (each row pair-duplicated along M so matmul output
    #                 lands on all 128 PSUM partitions -> no dup-DMA needed)
    #   cols 1024:1032  bvec^T as [o%128, o//128]
    #   cols 1032:1160  128x128 identity (for PE transposes in the softmax)
    setup_p = nc.declare_dram_parameter("setup", [128, 1160], F32, isOutput=False)
    enc_p = nc.declare_dram_parameter("enc", [LS * B, H], F32, isOutput=False)
    w_p = nc.declare_dram_parameter("w", [H, H], F32, isOutput=False)
    out_p = nc.declare_dram_parameter("out", [B, LS], F32, isOutput=True)

    # NOTE: must be built as bacc.Bacc + nc.compile() -- the staged walrus
    # rejects multi-wait instructions emitted by raw Bass+Tile; bacc
    # legalizes them. tensor_tensor_reduce does not encode with this walrus
    # ("ISA wrong length"), so the fused dot product uses
    # scalar_tensor_tensor(out, in0, 1.0, in1, mult, mult, accum_out).
    with tile.TileContext(nc) as tc:
        with (
            tc.tile_pool(name="const", bufs=1) as cp,
            tc.tile_pool(name="stream", bufs=8) as sp,
            tc.tile_pool(name="ps1", bufs=1, space="PSUM") as pp1,
            tc.tile_pool(name="psu", bufs=1, space="PSUM") as ppu,
        ):
            # ---- input DMAs (no deps; setup first so the PE chain is not
            # gated behind the 4 MB W transfer in the FIFO) ----
            setup = cp.tile([128, 1160], F32)
            # ACT HWDGE ring: posts in parallel with SP posting the W chunks
            nc.scalar.dma_start(setup[:], setup_p.ap())
            hT2 = setup[:, 0:1024]      # chunk k at [:, 128k:128k+128]
            bvT = setup[:, 1024:1032]
            idn = setup[:, 1032:1160]
            # W in 16 quarter-chunk DMAs ordered n-major: the (n, k) matmul
            # below needs only cols [512n:512n+512] of o-chunk k, so the
            # whole n=0 matmul group starts ~6us earlier than with full
            # 512KB chunk transfers
            wt = cp.tile([128, 8 * H], F32)  # W[o,h] as [o%128, (o//128, h)]
            for n in range(2):
                for k in range(8):
                    nc.sync.dma_start(
                        wt[:, 1024 * k + 512 * n : 1024 * k + 512 * n + 512],
                        w_p.ap()[128 * k : 128 * (k + 1), 512 * n : 512 * (n + 1)],
                    )

            # ---- u = hidden @ W on both b-halves ([128, 1024] in PSUM);
            # each 512-col half is copied out while the other half's matmuls
            # still run, so only the last half-copy sits on the critical path
            psum_u = ppu.tile([128, H], F32, tag="psum_u")
            u2 = cp.tile([128, H], F32)
            for n in range(2):
                for k in range(8):
                    nc.tensor.matmul(
                        psum_u[:, 512 * n : 512 * (n + 1)],
                        lhsT=hT2[:, 128 * k : 128 * (k + 1)],
                        rhs=wt[:, 1024 * k + 512 * n : 1024 * k + 512 * n + 512],
                        start=(k == 0),
                        stop=(k == 7),
                    )
                nc.scalar.copy(
                    u2[:, 512 * n : 512 * (n + 1)],
                    psum_u[:, 512 * n : 512 * (n + 1)],
                )

            # ---- c[b] = hidden[b] . bvec (both b-halves via dup'd hT2) ----
            psum_c = ppu.tile([128, 1], F32, tag="psum_c")
            for k in range(8):
                nc.tensor.matmul(
                    psum_c[:],
                    lhsT=hT2[:, 128 * k : 128 * (k + 1)],
                    rhs=bvT[:, k : k + 1],
                    start=(k == 0),
                    stop=(k == 7),
                )
            c2 = cp.tile([128, 1], F32)
            nc.scalar.copy(c2[:], psum_c[:])

            # ---- main stream: raw energies (no bias) per column ----
            enc_flat = enc_p.ap()  # [8192, 1024]
            ecols = cp.tile([128, NT * QK], F32)  # [128, 64]
            # persistent scratch: WAW between STTs stays program-order
            scr = cp.tile([128, 1], F32)
            for t in range(NT - 1):
                et = sp.tile([128, QK * H], F32, tag="et")
                src = enc_flat[512 * t : 512 * (t + 1)].rearrange(
                    "(q p) h -> p q h", p=128
                )
                nc.sync.dma_start(
                    et[:].rearrange("p (q h) -> p q h", q=QK), src
                )
                for q in range(QK):
                    c = QK * t + q
                    nc.vector.scalar_tensor_tensor(
                        out=scr[:].broadcast_to((128, H)),
                        in0=et[:, H * q : H * (q + 1)],
                        scalar=1.0,
                        in1=u2[:],
                        op0=MULT,
                        op1=MULT,
                        accum_out=ecols[:, c : c + 1],
                    )
            for h2 in range(2):  # final tile in two 1 MiB pieces
                et = sp.tile([128, 2 * H], F32, tag="et")
                base = 512 * (NT - 1) + 256 * h2
                src = enc_flat[base : base + 256].rearrange(
                    "(q p) h -> p q h", p=128
                )
                nc.sync.dma_start(
                    et[:].rearrange("p (q h) -> p q h", q=2), src
                )
                for q in range(2):
                    c = QK * (NT - 1) + 2 * h2 + q
                    nc.vector.scalar_tensor_tensor(
                        out=scr[:].broadcast_to((128, H)),
                        in0=et[:, H * q : H * (q + 1)],
                        scalar=1.0,
                        in1=u2[:],
                        op0=MULT,
                        op1=MULT,
                        accum_out=ecols[:, c : c + 1],
                    )

            # ---- add bias term: E += c[b] (per-partition scalar) ----
            ecols2 = cp.tile([128, NT * QK], F32)
            nc.vector.tensor_scalar_add(ecols2[:], ecols[:], c2[:, 0:1])

            # ---- softmax over b (per l), one chain per l-parity half ----
            out_sb = cp.tile([B, LS], F32)
            ov = out_sb[:].rearrange("b (l two) -> b two l", two=2)
            for lp in range(2):
                psum_t = pp1.tile([B, B], F32, tag=f"pt{lp}")
                nc.tensor.transpose(
                    psum_t[:],
                    ecols2[lp * B : (lp + 1) * B, :],
                    idn[lp * B : (lp + 1) * B, lp * B : (lp + 1) * B],
                )
                negm = cp.tile([B, 1], F32, tag=f"negm{lp}")
                nc.vector.tensor_reduce(
                    out=negm[:],
                    in_=psum_t[:],
                    axis=mybir.AxisListType.X,
                    op=mybir.AluOpType.max,
                    negate=True,
                )
                pexp = cp.tile([B, B], F32, tag=f"pexp{lp}")
                ssum = cp.tile([B, 1], F32, tag=f"ssum{lp}")
                nc.scalar.activation(
                    pexp[:],
                    psum_t[:],
                    mybir.ActivationFunctionType.Exp,
                    bias=negm[:, 0:1],
                    scale=1.0,
                    accum_out=ssum[:],
                )
                rs = cp.tile([B, 1], F32, tag=f"rs{lp}")
                nc.vector.reciprocal(rs[:], ssum[:])
                attn = cp.tile([B, B], F32, tag=f"attn{lp}")
                nc.vector.tensor_scalar_mul(attn[:], pexp[:], rs[:, 0:1])

                # back to [b, l-half], interleave into even/odd l slots
                psum_o = pp1.tile([B, B], F32, tag="po")
                nc.tensor.transpose(psum_o[:], attn[:], idn[0:B, 0:B])
                nc.vector.tensor_copy(ov[:, lp, :], psum_o[:])
            nc.sync.dma_start(out_p.ap(), out_sb[:])

    nc.compile()
    return nc


_IDENT = np.eye(128, dtype=np.float32)
_NC_CACHE = []


def _get_nc() -> bacc.Bacc:
    if not _NC_CACHE:
        _NC_CACHE.append(build_program())
    return _NC_CACHE[0]


def make_in_maps(hidden, encoder_outputs, W, b):
    # pure layout transforms (transpose/reshape/concat) done host-side so
    # the device skips on-chip transposes; all FLOPs stay on device
    hidden = np.asarray(hidden, dtype=np.float32)
    W = np.ascontiguousarray(W, dtype=np.float32)
    hidT2 = np.concatenate([hidden.T, hidden.T], axis=1)  # [H, 2B]
    # chunk k rows -> [p, (k, m)] so setup[:, 128k:128k+128] is lhsT chunk k
    hidT2p = hidT2.reshape(8, 128, 128).transpose(1, 0, 2).reshape(128, 1024)
    bvT = np.asarray(b, dtype=np.float32).reshape(8, 128).T  # [128, 8]
    setup = np.ascontiguousarray(
        np.concatenate([hidT2p, bvT, _IDENT], axis=1), dtype=np.float32
    )
    in_maps = []
    for i in range(NCORES):
        shard = np.ascontiguousarray(
            encoder_outputs[i * LS : (i + 1) * LS], dtype=np.float32
        ).reshape(LS * B, H)
        in_maps.append({"setup": setup, "enc": shard, "w": W})
    return in_maps


def kernel(hidden, encoder_outputs, W, b):
    nc = _get_nc()
    in_maps = make_in_maps(hidden, encoder_outputs, W, b)
    res = run_bass_kernel_spmd(nc, in_maps, core_ids=list(range(NCORES)))
    out = np.concatenate([res.results[i]["out"] for i in range(NCORES)], axis=1)
    return out[:, None, :].astype(np.float32)



# revision 2
# speedup vs baseline: 1.1447x; 1.1447x over previous
"""Trainium2 Bass kernel for nn_Attention (general-score attention energies +
softmax over the batch axis).

Math (reference):
    proj     = einsum('lbh,oh->lbo', enc, W) + b      # [L, B, H]
    energies = einsum('bh,lbh->bl', hidden, proj)     # [B, L]
    attn     = softmax(energies, axis=0)[:, None, :]  # [B, 1, L]

Algebraic rewrite used here:
    energies[b, l] = (hidden @ W)[b] . enc[l, b] + hidden[b] . b
This removes the O(L*B*H*H) projection matmul entirely; the kernel is a
memory-bound stream over enc with a tiny [B,H]x[H,H] matmul up front.

fp16 streaming: hidden/W/enc are downcast to fp16 host-side during the
shard/relayout pass, halving the dominant HBM stream (33.5 MB -> 16.8 MB
per core). All accumulation stays fp32 (PE PSUM accumulate; DVE free-dim
accumulator is fp32 regardless of operand dtype), and the softmax runs in
fp32. Measured output rel err vs the fp32 reference ~1.6e-3 (energies get
~9e-3 absolute noise on a sigma=24 logit scale; softmax over b is
near-one-hot and insensitive).

Distribution: enc is sharded along L across 8 cores (128 l-values per
core). The softmax is over the batch axis (per l), so every core's softmax
is fully local -- no collectives. hidden / W / b are replicated.

Per-core dataflow:
  - enc shard is relaid out host-side to [128, 64*1024] fp16 where
    partition p = (l%2)*64 + b and column c*1024+h holds enc[2c+l%2, b, h]:
    every DMA tile is a plain 2D slice with 16 KiB contiguous per
    partition -- max descriptor efficiency, no on-chip rearranges.
  - u = hidden @ W on TensorE in fp16 (PSUM fp32), downcast to fp16 in
    SBUF. hidden^T arrives pre-transposed and column-duplicated so the
    matmul output covers all 128 PSUM partitions (both l-parity b-halves).
  - stream enc in 2 MiB tiles [128, 8*1024] fp16; one DVE
    scalar_tensor_tensor per [128, 1024] block computes
    ecol = sum_h enc*u in a single pass (fp32 accum_out); all operands
    fp16 + packed so DVE runs in 2x mode. Bias c added once at the end.
  - softmax over b: PE transpose [64,64] halves -> reduce_max(negate) ->
    ScalarE exp(+bias) with fused row-sum -> reciprocal -> scale ->
    PE transpose back, interleave even/odd l, DMA out [64, 128] fp32.

Timing notes: DMA-bound; ~19.2 MB/core (16.8 enc + 2.1 W + 0.3 setup) at
~330-360 GB/s steady-state + ~10 us NEFF startup + short softmax tail.
"""

import numpy as np

import concourse.bass as bass
import concourse.bacc as bacc
import concourse.tile as tile
from concourse import mybir
from concourse.bass_utils import run_bass_kernel_spmd

F32 = mybir.dt.float32
F16 = mybir.dt.float16

B = 64          # batch
H = 1024        # hidden dim
L = 1024        # enc_len
NCORES = 8
LS = L // NCORES            # 128 l-values per core
C = LS // 2                 # 64 (l-pair) column blocks per core
TILE_C = 8                  # column blocks per DMA tile (2 MiB fp16)
NT = C // TILE_C            # 8 stream tiles per core
MULT = mybir.AluOpType.mult
ADD = mybir.AluOpType.add


def build_program() -> bacc.Bacc:
    nc = bacc.Bacc(
        "TRN2", target_bir_lowering=False, debug=False, num_devices=NCORES
    )

    # setup16 packs [hidT duplicated to both b-halves | bvT], fp16:
    #   cols 0:1024   hidden^T chunks: [:, 128k:128k+128] = lhsT for o-chunk k
    #                 (rows pair-duplicated along M so the matmul output
    #                 lands on all 128 PSUM partitions -> no dup copies)
    #   cols 1024:1032  bvec^T as [o%128, o//128]
    setup_p = nc.declare_dram_parameter("setup16", [128, 1032], F16, isOutput=False)
    idn_p = nc.declare_dram_parameter("idn", [128, 128], F32, isOutput=False)
    # enc fp16, partition-contiguous: [p = (l%2)*64+b, c*1024+h]
    enc_p = nc.declare_dram_parameter("enc", [128, C * H], F16, isOutput=False)
    # W fp16 as [p, n*4096 + k*512 + j] = W[o=128k+p, h=512n+j]
    w_p = nc.declare_dram_parameter("w", [128, 8 * H], F16, isOutput=False)
    out_p = nc.declare_dram_parameter("out", [B, LS], F32, isOutput=True)

    # NOTE: must be built as bacc.Bacc + nc.compile() -- the staged walrus
    # rejects multi-wait instructions emitted by raw Bass+Tile; bacc
    # legalizes them. tensor_tensor_reduce does not encode with this walrus
    # ("ISA wrong length"), so the fused dot product uses
    # scalar_tensor_tensor(out, in0, 1.0, in1, mult, mult, accum_out).
    with tile.TileContext(nc) as tc:
        with (
            tc.tile_pool(name="const", bufs=1) as cp,
            tc.tile_pool(name="stream", bufs=6) as sp,
            tc.tile_pool(name="ps1", bufs=1, space="PSUM") as pp1,
            tc.tile_pool(name="psu", bufs=1, space="PSUM") as ppu,
        ):
            # ---- input DMAs: setup/idn/W on the ACT HWDGE ring so the
            # sync ring carries nothing but the enc stream ----
            setup = cp.tile([128, 1032], F16)
            nc.scalar.dma_start(setup[:], setup_p.ap())
            hT2 = setup[:, 0:1024]      # chunk k at [:, 128k:128k+128]
            bvT = setup[:, 1024:1032]
            idn = cp.tile([128, 128], F32)
            nc.scalar.dma_start(idn[:], idn_p.ap())
            # W n-halves: the (n, k) matmul group below needs only
            # [:, 4096n:4096n+4096], so n=0 work starts at half transfer
            wt = cp.tile([128, 8 * H], F16)
            for n in range(2):
                nc.scalar.dma_start(
                    wt[:, 4096 * n : 4096 * (n + 1)],
                    w_p.ap()[:, 4096 * n : 4096 * (n + 1)],
                )

            # ---- u = hidden @ W on both b-halves ([128, 1024] in PSUM);
            # fp16 inputs, fp32 accumulate, downcast to fp16 in SBUF
            psum_u = ppu.tile([128, H], F32, tag="psum_u")
            u2 = cp.tile([128, H], F16)
            for n in range(2):
                for k in range(8):
                    nc.tensor.matmul(
                        psum_u[:, 512 * n : 512 * (n + 1)],
                        lhsT=hT2[:, 128 * k : 128 * (k + 1)],
                        rhs=wt[:, 4096 * n + 512 * k : 4096 * n + 512 * (k + 1)],
                        start=(k == 0),
                        stop=(k == 7),
                    )
                nc.scalar.copy(
                    u2[:, 512 * n : 512 * (n + 1)],
                    psum_u[:, 512 * n : 512 * (n + 1)],
                )

            # ---- c[b] = hidden[b] . bvec (both b-halves via dup'd hT2) ----
            psum_c = ppu.tile([128, 1], F32, tag="psum_c")
            for k in range(8):
                nc.tensor.matmul(
                    psum_c[:],
                    lhsT=hT2[:, 128 * k : 128 * (k + 1)],
                    rhs=bvT[:, k : k + 1],
                    start=(k == 0),
                    stop=(k == 7),
                )
            c2 = cp.tile([128, 1], F32)
            nc.scalar.copy(c2[:], psum_c[:])

            # ---- main stream: raw energies (no bias) per column ----
            ecols = cp.tile([128, C], F32)  # [128, 64]
            # real fp16 scratch out (not a broadcast view): all non-scalar
            # STT operands 2-byte + packed => DVE 2x_1p mode
            scr = cp.tile([128, H], F16)
            for t in range(NT):
                et = sp.tile([128, TILE_C * H], F16, tag="et")
                nc.sync.dma_start(
                    et[:], enc_p.ap()[:, TILE_C * H * t : TILE_C * H * (t + 1)]
                )
                for q in range(TILE_C):
                    c = TILE_C * t + q
                    nc.vector.scalar_tensor_tensor(
                        out=scr[:],
                        in0=et[:, H * q : H * (q + 1)],
                        scalar=1.0,
                        in1=u2[:],
                        op0=MULT,
                        op1=MULT,
                        accum_out=ecols[:, c : c + 1],
                    )

            # ---- add bias term: E += c[b] (per-partition scalar) ----
            ecols2 = cp.tile([128, C], F32)
            nc.vector.tensor_scalar_add(ecols2[:], ecols[:], c2[:, 0:1])

            # ---- softmax over b (per l), one chain per l-parity half ----
            out_sb = cp.tile([B, LS], F32)
            ov = out_sb[:].rearrange("b (l two) -> b two l", two=2)
            for lp in range(2):
                psum_t = pp1.tile([B, B], F32, tag=f"pt{lp}")
                nc.tensor.transpose(
                    psum_t[:],
                    ecols2[lp * B : (lp + 1) * B, :],
                    idn[lp * B : (lp + 1) * B, lp * B : (lp + 1) * B],
                )
                negm = cp.tile([B, 1], F32, tag=f"negm{lp}")
                nc.vector.tensor_reduce(
                    out=negm[:],
                    in_=psum_t[:],
                    axis=mybir.AxisListType.X,
                    op=mybir.AluOpType.max,
                    negate=True,
                )
                pexp = cp.tile([B, B], F32, tag=f"pexp{lp}")
                ssum = cp.tile([B, 1], F32, tag=f"ssum{lp}")
                nc.scalar.activation(
                    pexp[:],
                    psum_t[:],
                    mybir.ActivationFunctionType.Exp,
                    bias=negm[:, 0:1],
                    scale=1.0,
                    accum_out=ssum[:],
                )
                rs = cp.tile([B, 1], F32, tag=f"rs{lp}")
                nc.vector.reciprocal(rs[:], ssum[:])
                attn = cp.tile([B, B], F32, tag=f"attn{lp}")
                nc.vector.tensor_scalar_mul(attn[:], pexp[:], rs[:, 0:1])

                # back to [b, l-half], interleave into even/odd l slots
                psum_o = pp1.tile([B, B], F32, tag="po")
                nc.tensor.transpose(psum_o[:], attn[:], idn[0:B, 0:B])
                nc.vector.tensor_copy(ov[:, lp, :], psum_o[:])
            nc.sync.dma_start(out_p.ap(), out_sb[:])

    nc.compile()
    return nc


_IDENT = np.eye(128, dtype=np.float32)
_NC_CACHE = []


def _get_nc() -> bacc.Bacc:
    if not _NC_CACHE:
        _NC_CACHE.append(build_program())
    return _NC_CACHE[0]


def make_in_maps(hidden, encoder_outputs, W, b):
    # layout transforms + fp16 downcast done host-side during sharding;
    # all FLOPs (matmul, dot products, softmax) stay on device
    hidden16 = np.asarray(hidden, dtype=np.float16)
    hidT2 = np.concatenate([hidden16.T, hidden16.T], axis=1)  # [H, 2B]
    # chunk k rows -> [p, (k, m)] so setup[:, 128k:128k+128] is lhsT chunk k
    hidT2p = hidT2.reshape(8, 128, 128).transpose(1, 0, 2).reshape(128, 1024)
    bvT = np.asarray(b, dtype=np.float16).reshape(8, 128).T  # [128, 8]
    setup16 = np.ascontiguousarray(
        np.concatenate([hidT2p, bvT], axis=1), dtype=np.float16
    )
    # W[o, h] -> [p, (n, k, j)] = W[128k+p, 512n+j]
    W16 = np.asarray(W, dtype=np.float16)
    wt = np.ascontiguousarray(
        W16.reshape(8, 128, 2, 512).transpose(1, 2, 0, 3).reshape(128, 8192)
    )
    in_maps = []
    for i in range(NCORES):
        shard16 = encoder_outputs[i * LS : (i + 1) * LS].astype(np.float16)
        # [l, b, h] -> [p = (l%2)*64 + b, (c, h)] with l = 2c + l%2
        enc_pc = np.ascontiguousarray(
            shard16.reshape(C, 2, B, H).transpose(1, 2, 0, 3).reshape(128, C * H)
        )
        in_maps.append(
            {"setup16": setup16, "idn": _IDENT, "enc": enc_pc, "w": wt}
        )
    return in_maps


def kernel(hidden, encoder_outputs, W, b):
    nc = _get_nc()
    in_maps = make_in_maps(hidden, encoder_outputs, W, b)
    res = run_bass_kernel_spmd(nc, in_maps, core_ids=list(range(NCORES)))
    out = np.concatenate([res.results[i]["out"] for i in range(NCORES)], axis=1)
    return out[:, None, :].astype(np.float32)


# revision 3
# speedup vs baseline: 2.1652x; 1.8914x over previous
"""Trainium2 Bass kernel for nn_Attention (general-score attention energies +
softmax over the batch axis).

Math (reference):
    proj     = einsum('lbh,oh->lbo', enc, W) + b      # [L, B, H]
    energies = einsum('bh,lbh->bl', hidden, proj)     # [B, L]
    attn     = softmax(energies, axis=0)[:, None, :]  # [B, 1, L]

Algebraic rewrite:
    energies[b, l] = (hidden @ W)[b] . enc[l, b] + hidden[b] . b
This removes the O(L*B*H*H) projection matmul; the kernel is a memory-bound
stream over enc with a tiny [B,H]x[H,H] matmul up front.

fp16 streaming: hidden/W/enc are downcast to fp16 host-side during the
shard/relayout pass, halving the dominant HBM stream (33.5 MB -> 16.8 MB
per core). All accumulation is fp32 (PE PSUM accumulate); the softmax runs
in fp32. Output rel err vs the fp32 reference ~1.6e-3 (2e-2 gate).

Distribution: enc sharded along L across 8 cores (128 l-values per core).
Softmax is over batch (per l) => fully core-local, no collectives.
hidden / W / b replicated.

Per-core dataflow (v3 -- h-contraction on the TensorEngine):
  - enc shard is relaid out host-side to [128, 16*8*64*8] fp16 with
    partition p = h%128 and free = (tile, h-chunk, l, b): every DMA tile is
    a plain 2D slice, 8 KiB contiguous per partition.
  - uT[h, b] = sum_o W[o, h] hidden[b, o] computed directly transposed on
    PE (64 fp16 matmuls W-chunk^T @ hidden-chunk^T, fp32 PSUM), downcast
    to fp16.
  - per tile (8 l-values): 8 accumulating matmuls
    uT_k^T [128h, 64b] @ enc_k [128h, (8l, 64b')] -> PSUM [64b, (8l, 64b')].
    The energies are the b-diagonal of each [64, 64] block: one small DVE
    scalar_tensor_tensor per l against an fp32 identity with fused
    accum_out (192 ns each) extracts E[b, l] = sum_b' P[b, l, b']*I[b, b'].
    (A direct DVE dot-product needs scalar_tensor_tensor over [128, 1024]
    at 1127 ns/l -- the fused-accum DVE ops have no 2x mode -- which made
    DVE the bottleneck; PE does the contraction 5x cheaper.)
  - softmax over b: single chain, PE transpose [64,128] -> [128 l, 64 b],
    reduce_max(negate) -> ScalarE exp(+bias) with fused row-sum ->
    reciprocal -> scale -> PE transpose back -> DMA out [64, 128] fp32.

All pre-enc inputs (setup/W/idn, 2.3 MB) go FIRST on the same sync-ring
queue as enc so nothing gates on the slow scalar ring (v2 lost 25 us to W
on the scalar ring at ~97 GB/s while enc hogged the sync ring).

Engine budget per core: DMA 19.1 MB (~44-53 us at 360-430 GB/s), PE ~27 us,
DVE ~25 us, all overlapped => DMA-bound.
"""

import numpy as np

import concourse.bass as bass
import concourse.bacc as bacc
import concourse.tile as tile
from concourse import mybir
from concourse.bass_utils import run_bass_kernel_spmd

F32 = mybir.dt.float32
F16 = mybir.dt.float16

B = 64          # batch
H = 1024        # hidden dim
L = 1024        # enc_len
NCORES = 8
LS = L // NCORES            # 128 l-values per core
TILE_L = 8                  # l-values per stream tile (2 MiB fp16)
NT = LS // TILE_L           # 16 stream tiles per core
KH = H // 128               # 8 h-chunks (PE contraction dim)
MULT = mybir.AluOpType.mult
ADD = mybir.AluOpType.add


def build_program() -> bacc.Bacc:
    nc = bacc.Bacc(
        "TRN2", target_bir_lowering=False, debug=False, num_devices=NCORES
    )

    # st (fp16): cols 0:512 hidden^T chunks (st[p, 64ko+b] = hidden[b, 128ko+p])
    #            cols 512:520 bvec^T (st[p, 512+ko] = bvec[128ko+p])
    st_p = nc.declare_dram_parameter("st", [128, 520], F16, isOutput=False)
    # W (fp16): wt[p, 1024kh + 128ko + j] = W[o=128ko+p, h=128kh+j]
    w_p = nc.declare_dram_parameter("w", [128, 8 * H], F16, isOutput=False)
    idn_p = nc.declare_dram_parameter("idn", [128, 128], F32, isOutput=False)
    # enc (fp16): enc[p, 4096t + 512k + 64c + b] = enc_shard[l=8t+c, b, h=128k+p]
    enc_p = nc.declare_dram_parameter("enc", [128, NT * 4096], F16, isOutput=False)
    out_p = nc.declare_dram_parameter("out", [B, LS], F32, isOutput=True)

    # NOTE: built as bacc.Bacc + nc.compile() -- the staged walrus rejects
    # multi-wait instructions emitted by raw Bass+Tile; bacc legalizes them.
    with tile.TileContext(nc) as tc:
        with (
            tc.tile_pool(name="const", bufs=1) as cp,
            tc.tile_pool(name="stream", bufs=6) as sp,
            tc.tile_pool(name="pse", bufs=3, space="PSUM") as ppe,
            tc.tile_pool(name="ps1", bufs=1, space="PSUM") as pp1,
            tc.tile_pool(name="psu", bufs=1, space="PSUM") as ppu,
        ):
            # ---- pre-enc inputs, all first on the sync ring ----
            st = cp.tile([128, 520], F16)
            nc.sync.dma_start(st[:], st_p.ap())
            hT = st[:, 0:512]           # chunk ko at [:, 64ko : 64ko+64]
            bvT = st[:, 512:520]
            wt = cp.tile([128, 8 * H], F16)
            for h2 in range(2):
                nc.sync.dma_start(
                    wt[:, 4096 * h2 : 4096 * (h2 + 1)],
                    w_p.ap()[:, 4096 * h2 : 4096 * (h2 + 1)],
                )
            idn = cp.tile([128, 128], F32)
            nc.sync.dma_start(idn[:], idn_p.ap())

            # ---- uT[h, b] on PE: accumulate over o-chunks, fp32 PSUM ----
            psum_ut = ppu.tile([128, 512], F32, tag="psum_ut")
            for kh in range(KH):
                for ko in range(8):
                    nc.tensor.matmul(
                        psum_ut[:, 64 * kh : 64 * (kh + 1)],
                        lhsT=wt[:, 1024 * kh + 128 * ko : 1024 * kh + 128 * (ko + 1)],
                        rhs=hT[:, 64 * ko : 64 * (ko + 1)],
                        start=(ko == 0),
                        stop=(ko == 7),
                    )
            uT = cp.tile([128, 512], F16)   # uT[p, 64k+b] = u[b, 128k+p]
            nc.scalar.copy(uT[:], psum_ut[:])

            # ---- c[b] = hidden[b] . bvec ----
            psum_c = ppu.tile([B, 1], F32, tag="psum_c")
            for ko in range(8):
                nc.tensor.matmul(
                    psum_c[:],
                    lhsT=hT[:, 64 * ko : 64 * (ko + 1)],
                    rhs=bvT[:, ko : ko + 1],
                    start=(ko == 0),
                    stop=(ko == 7),
                )
            c2 = cp.tile([B, 1], F32)
            nc.scalar.copy(c2[:], psum_c[:])

            # ---- main stream: PE contraction + DVE diagonal extract ----
            ecols = cp.tile([B, LS], F32)
            scr = cp.tile([B, B], F32)   # diag STT main-out scratch
            for t in range(NT):
                et = sp.tile([128, 4096], F16, tag="et")
                nc.sync.dma_start(
                    et[:], enc_p.ap()[:, 4096 * t : 4096 * (t + 1)]
                )
                pe_t = ppe.tile([B, 512], F32, tag="pe")
                for k in range(KH):
                    nc.tensor.matmul(
                        pe_t[:],
                        lhsT=uT[:, 64 * k : 64 * (k + 1)],
                        rhs=et[:, 512 * k : 512 * (k + 1)],
                        start=(k == 0),
                        stop=(k == KH - 1),
                    )
                for c in range(TILE_L):
                    l = TILE_L * t + c
                    nc.vector.scalar_tensor_tensor(
                        out=scr[:],
                        in0=pe_t[:, B * c : B * (c + 1)],
                        scalar=1.0,
                        in1=idn[0:B, 0:B],
                        op0=MULT,
                        op1=MULT,
                        accum_out=ecols[:, l : l + 1],
                    )

            # ---- bias, then softmax over b (single chain) ----
            ecols2 = cp.tile([B, LS], F32)
            nc.vector.tensor_scalar_add(ecols2[:], ecols[:], c2[:, 0:1])

            psum_t = pp1.tile([LS, B], F32, tag="pt")
            nc.tensor.transpose(psum_t[:], ecols2[:], idn[0:B, 0:B])
            negm = cp.tile([LS, 1], F32)
            nc.vector.tensor_reduce(
                out=negm[:],
                in_=psum_t[:],
                axis=mybir.AxisListType.X,
                op=mybir.AluOpType.max,
                negate=True,
            )
            pexp = cp.tile([LS, B], F32)
            ssum = cp.tile([LS, 1], F32)
            nc.scalar.activation(
                pexp[:],
                psum_t[:],
                mybir.ActivationFunctionType.Exp,
                bias=negm[:, 0:1],
                scale=1.0,
                accum_out=ssum[:],
            )
            rs = cp.tile([LS, 1], F32)
            nc.vector.reciprocal(rs[:], ssum[:])
            attn = cp.tile([LS, B], F32)
            nc.vector.tensor_scalar_mul(attn[:], pexp[:], rs[:, 0:1])

            psum_o = pp1.tile([B, LS], F32, tag="po")
            nc.tensor.transpose(psum_o[:], attn[:], idn[:, :])
            out_sb = cp.tile([B, LS], F32)
            nc.vector.tensor_copy(out_sb[:], psum_o[:])
            nc.sync.dma_start(out_p.ap(), out_sb[:])

    nc.compile()
    return nc


_IDENT = np.eye(128, dtype=np.float32)
_NC_CACHE = []


def _get_nc() -> bacc.Bacc:
    if not _NC_CACHE:
        _NC_CACHE.append(build_program())
    return _NC_CACHE[0]


def make_in_maps(hidden, encoder_outputs, W, b):
    # layout transforms + fp16 downcast done host-side during sharding;
    # all FLOPs (matmuls, energy contraction, softmax) stay on device
    hidden16 = np.asarray(hidden, dtype=np.float16)
    # st[p, 64ko + b] = hidden[b, 128ko + p]
    hTp = hidden16.T.reshape(8, 128, B).transpose(1, 0, 2).reshape(128, 512)
    bvT = np.asarray(b, dtype=np.float16).reshape(8, 128).T  # [128, 8]
    st = np.ascontiguousarray(np.concatenate([hTp, bvT], axis=1))
    # wt[p, 1024kh + 128ko + j] = W[128ko + p, 128kh + j]
    W16 = np.asarray(W, dtype=np.float16)
    wt = np.ascontiguousarray(
        W16.reshape(8, 128, 8, 128).transpose(1, 2, 0, 3).reshape(128, 8192)
    )
    in_maps = []
    for i in range(NCORES):
        shard16 = encoder_outputs[i * LS : (i + 1) * LS].astype(np.float16)
        # [l, b, h] -> [p = h%128, (t, k, c, b)] with l = 8t+c, h = 128k+p
        enc_pc = np.ascontiguousarray(
            shard16.reshape(NT, TILE_L, B, KH, 128)
            .transpose(4, 0, 3, 1, 2)
            .reshape(128, NT * 4096)
        )
        in_maps.append({"st": st, "idn": _IDENT, "enc": enc_pc, "w": wt})
    return in_maps


def kernel(hidden, encoder_outputs, W, b):
    nc = _get_nc()
    in_maps = make_in_maps(hidden, encoder_outputs, W, b)
    res = run_bass_kernel_spmd(nc, in_maps, core_ids=list(range(NCORES)))
    out = np.concatenate([res.results[i]["out"] for i in range(NCORES)], axis=1)
    return out[:, None, :].astype(np.float32)
